# revision 19
# baseline (speedup 1.0000x reference)
"""CrystalGNN (GCNConv + mean-pool + FC + log_softmax) on 8 TRN2 NeuronCores.

Strategy (dst-range partitioned, dense normalized adjacency):
- Core c owns dst nodes [c*1250, (c+1)*1250). The host builds the
  normalized adjacency block A_c[src, dst_local] = 16 * sum over edges
  (incl. self-loops) of dinv[src]*dinv[dst], shipped as fp8-e4m3
  (~12.6MB/core), pre-blocked bank-major (per 512-col PSUM bank, all 80
  src blocks, partition = src % 128). The x16 scale keeps norms in
  e4m3's sweet spot; it is undone exactly via bias*16 + relu + pool/16.
- Device: h = x@W (fp8) on PE, kept in SBUF; out^T[H, dst] accumulates
  DoubleRow fp8 matmuls (2 src blocks per instruction) over 40 block
  pairs, one dst bank at a time, so each bank's post-processing (fused
  relu+bias ACT drain, PE transpose, pooling matmul) overlaps the next
  bank's A-stream. A-stream DMAs round-robin over three DGE queues.
- Each core outputs its partial (pre-softmax) logits [64,2] = FC(its
  pooled partial) + b_fc/8; there is no cross-core communication on
  device. kernel() gathers the 8 partial outputs and unshards (sums +
  log_softmax) on host, like any other sum-sharded output gather.
"""
import numpy as np
import ml_dtypes

N = 10000
E = 640000
F = 128
HD = 128
G = 64
NC = 8
PER = N // NC           # 1250
NPAD = 10240            # 80 blocks of 128 src rows (even, for DoubleRow pairs)
NBLK = NPAD // 128      # 80
BANK = 512
BANKW = [512, 512, 226]
NBANK = 3
ABLK = 16               # src blocks per A-stream DMA
ASCALE = 16.0

BF16 = ml_dtypes.bfloat16
F8 = ml_dtypes.float8_e4m3

def _plan(edge_index, batch_idx):
    src = edge_index[0].astype(np.int64)
    dst = edge_index[1].astype(np.int64)
    loops = np.arange(N, dtype=np.int64)
    src_f = np.concatenate([src, loops])
    dst_f = np.concatenate([dst, loops])

    deg = np.bincount(dst_f, minlength=N).astype(np.float64)
    dinv = 1.0 / np.sqrt(deg)
    wts = dinv[src_f] * dinv[dst_f] * ASCALE

    core_of = dst_f // PER
    A_ship = np.zeros((NC, 128, NBLK * PER), dtype=F8)
    for c in range(NC):
        m = core_of == c
        flat = src_f[m] * PER + (dst_f[m] - c * PER)
        A = np.bincount(flat, weights=wts[m], minlength=NPAD * PER)
        A3 = A.reshape(NBLK, 128, PER).transpose(1, 0, 2)  # [128, NBLK, PER]
        off = 0
        for o0, w in zip((0, 512, 1024), BANKW):
            A_ship[c, :, off:off + NBLK * w] = \
                A3[:, :, o0:o0 + w].reshape(128, NBLK * w).astype(F8)
            off += NBLK * w

    cnt = np.bincount(batch_idx.astype(np.int64), minlength=G).astype(np.float64)
    cnt = np.maximum(cnt, 1.0)
    mp = np.zeros((NC, 1280, G), dtype=np.float64)
    for c in range(NC):
        nodes = np.arange(c * PER, (c + 1) * PER)
        g = batch_idx[nodes].astype(np.int64)
        mp[c, np.arange(PER), g] = 1.0 / (cnt[g] * ASCALE)
    mp = mp.reshape(NC, 10, 128, G)
    mp = np.transpose(mp, (0, 2, 1, 3)).reshape(NC, 128, 10 * G).astype(np.float32)

    return dict(A_ship=A_ship, mpool=mp)


def _build():
    import concourse.bacc as bacc
    import concourse.mybir as mybir
    import concourse.tile as tile

    f32 = mybir.dt.float32
    fp8 = mybir.dt.float8e4
    AF = mybir.ActivationFunctionType
    ALU = mybir.AluOpType
    DR = mybir.MatmulPerfMode.DoubleRow

    nc = bacc.Bacc("TRN2", target_bir_lowering=False, debug=False, num_devices=NC)

    xT = nc.dram_tensor("xT", [F, NPAD], fp8, kind="ExternalInput")
    Wt = nc.dram_tensor("Wt", [F, HD], fp8, kind="ExternalInput")
    Amat = nc.dram_tensor("Amat", [128, NBLK * PER], fp8, kind="ExternalInput")
    bvec = nc.dram_tensor("bvec", [128, 1], f32, kind="ExternalInput")  # 16*b
    Wfc = nc.dram_tensor("Wfc", [HD, 2], f32, kind="ExternalInput")
    bfc = nc.dram_tensor("bfc", [G, 2], f32, kind="ExternalInput")      # b_fc/NC bcast
    idn = nc.dram_tensor("idn", [128, 128], f32, kind="ExternalInput")  # identity
    mpool = nc.dram_tensor("mpool", [128, 10 * G], f32, kind="ExternalInput")
    out = nc.dram_tensor("out", [G, 2], f32, kind="ExternalOutput")

    with tile.TileContext(nc) as tc:
        with tc.tile_pool(name="const", bufs=1) as cp, \
             tc.tile_pool(name="astream", bufs=3) as ap_pool, \
             tc.tile_pool(name="aggp", bufs=1, space="PSUM") as aggp, \
             tc.tile_pool(name="tps", bufs=2, space="PSUM") as tps:

            # ---- constants: W + xT on scalar queue (A-stream owns the
            # sync/gpsimd rings at startup), small consts after xt0 ----
            W_sb = cp.tile([F, HD], fp8)
            nc.scalar.dma_start(W_sb[:], Wt[:])
            XC = 16 * 128           # xT load chunk (16 src blocks)
            xt_tiles = []
            for xc in range(NBLK // 16):
                lo = xc * XC
                hi = min(NPAD, lo + XC)
                xt_t = cp.tile([F, hi - lo], fp8, name=f"xt{xc}")
                nc.scalar.dma_start(xt_t[:], xT[:, lo:hi])
                xt_tiles.append(xt_t)
            bv_sb = cp.tile([128, 1], f32)
            nc.scalar.dma_start(bv_sb[:], bvec[:])
            Wfc_sb = cp.tile([HD, 2], f32)
            nc.scalar.dma_start(Wfc_sb[:], Wfc[:])
            bfc_sb = cp.tile([G, 2], f32)
            nc.scalar.dma_start(bfc_sb[:], bfc[:])
            idn_sb = cp.tile([128, 128], f32)
            nc.scalar.dma_start(idn_sb[:], idn[:])
            mp_sb = cp.tile([128, 10 * G], f32)
            nc.scalar.dma_start(mp_sb[:], mpool[:])

            agg = []
            for bk in range(NBANK):
                agg_t = aggp.tile([128, BANK], f32, tag=f"agg{bk}", name=f"agg{bk}")
                agg.append(agg_t)

            # ---- main: bank 0 interleaves h = x@W group-by-group with its
            # A-tile consumption (PE fills DMA-bound slack with h work);
            # banks 1-2 are pure A streaming ----
            h_sb = cp.tile([128, NBLK * 128], fp8)
            outT_sb = cp.tile([128, 1280], f32)
            hn_sb = cp.tile([128, 1280], f32)
            pp = tps.tile([128, G], f32, tag="pool", name="pp", bufs=1)
            aoff = 0
            dma_engines = [nc.sync, nc.gpsimd, nc.scalar]
            dma_i = 0
            NPAIR = NBLK // 2

            def h_group(cb, hps):
                njj = min(4, NBLK - cb)
                hp = hps.tile([128, 512], f32, tag="hps")
                for j in range(njj):
                    ci = cb + j
                    xt_t = xt_tiles[ci // 16]
                    co = (ci % 16) * 128
                    nc.tensor.matmul(
                        hp[:, j * 128:(j + 1) * 128],
                        xt_t[:, co:co + 128],
                        W_sb[:],
                        start=True, stop=True,
                    )
                half = njj * 64
                nc.vector.tensor_copy(
                    h_sb[:, cb * 128:cb * 128 + half], hp[:, :half])
                nc.scalar.copy(
                    h_sb[:, cb * 128 + half:(cb + njj) * 128],
                    hp[:, half:njj * 128])

            with tc.tile_pool(name="hps", bufs=3, space="PSUM") as hps:
                for bk in range(NBANK):
                    w = BANKW[bk]
                    for b0 in range(0, NBLK, ABLK):
                        nb = min(ABLK, NBLK - b0)
                        at = ap_pool.tile([128, ABLK * BANK], fp8, tag="at",
                                          bufs=6)
                        eng = dma_engines[dma_i % len(dma_engines)]
                        dma_i += 1
                        eng.dma_start(
                            at[:, :nb * w],
                            Amat[:, aoff + b0 * w:aoff + (b0 + nb) * w])
                        if bk == 0:
                            # compute h for exactly the blocks this A tile
                            # contracts with, right before consuming it
                            for cb in range(b0, b0 + nb, 4):
                                h_group(cb, hps)
                        for j in range(0, nb, 2):
                            pr = (b0 + j) // 2
                            nc.tensor.matmul(
                                agg[bk][:, :w],
                                h_sb[:, (b0 + j) * 128:(b0 + j + 2) * 128]
                                    .rearrange("p (k m) -> p k m", k=2),
                                at[:, j * w:(j + 2) * w]
                                    .rearrange("p (k n) -> p k n", k=2),
                                start=(pr == 0),
                                stop=(pr == NPAIR - 1),
                                skip_group_check=True,
                                perf_mode=DR,
                            )
                    aoff += NBLK * w
                    # fused bias+relu drain (out^T: bias per-partition, x16)
                    nc.scalar.activation(
                        outT_sb[:, bk * BANK:bk * BANK + w], agg[bk][:, :w],
                        AF.Relu, bias=bv_sb[:, 0:1])
                    if w < BANK and bk * BANK + w < 1280:
                        nc.vector.memset(outT_sb[:, bk * BANK + w:1280], 0.0)
                    t0 = (bk * BANK) // 128
                    t1 = (bk * BANK + w + 127) // 128
                    for t in range(t0, min(t1, 10)):
                        tp = tps.tile([128, 128], f32, tag="tp", bufs=1)
                        nc.tensor.transpose(
                            tp[:], outT_sb[:, t * 128:(t + 1) * 128], idn_sb[:])
                        nc.vector.tensor_copy(
                            hn_sb[:, t * 128:(t + 1) * 128], tp[:])
                        nc.tensor.matmul(
                            pp[:],
                            hn_sb[:, t * 128:(t + 1) * 128],
                            mp_sb[:, t * G:(t + 1) * G],
                            start=(t == 0), stop=(t == 9),
                            skip_group_check=True,
                        )

            pooled_sb = cp.tile([128, G], f32)
            nc.scalar.copy(pooled_sb[:], pp[:])

            # ---- FC partial (+ b_fc/NC), straight to DRAM out ----
            lg = tps.tile([G, 2], f32, tag="pool", name="lg", bufs=1)
            nc.tensor.matmul(lg[:], pooled_sb[:], Wfc_sb[:], start=True, stop=True)
            lpart = cp.tile([G, 2], f32)
            nc.vector.tensor_tensor(lpart[:], lg[:], bfc_sb[:], ALU.add)
            nc.sync.dma_start(out[:], lpart[:])

    nc.compile()
    return nc


def _make_inputs(x, W, b, W_fc, b_fc, p):
    xT = np.zeros((F, NPAD), dtype=F8)
    xT[:, :N] = np.asarray(x, dtype=np.float32).T.astype(F8)
    shared = dict(
        xT=xT,
        Wt=np.asarray(W, dtype=np.float32).astype(F8),
        bvec=(np.asarray(b, dtype=np.float32) * ASCALE).reshape(128, 1).copy(),
        Wfc=np.asarray(W_fc, dtype=np.float32),
        bfc=np.tile(np.asarray(b_fc, dtype=np.float32)[None, :] / NC, (G, 1)),
        idn=np.eye(128, dtype=np.float32),
    )
    in_maps = []
    for c in range(NC):
        m = dict(shared)
        m["Amat"] = p["A_ship"][c]
        m["mpool"] = p["mpool"][c]
        in_maps.append(m)
    return in_maps


def _post(results):
    """Gather/unshard: sum the 8 partial logits, then log_softmax."""
    logits = np.zeros((G, 2), dtype=np.float64)
    for r in results:
        logits += np.asarray(r["out"], dtype=np.float64)
    mx = logits.max(axis=1, keepdims=True)
    e = np.exp(logits - mx)
    ls = logits - mx - np.log(e.sum(axis=1, keepdims=True))
    return ls.astype(np.float32)


def kernel(x, edge_index, batch_idx, W, b, W_fc, b_fc):
    from concourse.bass_utils import run_bass_kernel_spmd

    p = _plan(np.asarray(edge_index), np.asarray(batch_idx))
    nc = _build()
    in_maps = _make_inputs(x, W, b, W_fc, b_fc, p)
    res = run_bass_kernel_spmd(nc, in_maps, core_ids=list(range(NC)))
    return _post(res.results)


# revision 21
# speedup vs baseline: 1.0209x; 1.0209x over previous
"""CrystalGNN (GCNConv + mean-pool + FC + log_softmax) on 8 TRN2 NeuronCores.

Strategy (dst-range partitioned, dense normalized adjacency):
- Core c owns dst nodes [c*1250, (c+1)*1250). The host builds the
  normalized adjacency block A_c[src, dst_local] = 16 * sum over edges
  (incl. self-loops) of dinv[src]*dinv[dst], shipped as fp8-e4m3
  (~12.6MB/core), pre-blocked bank-major (per 512-col PSUM bank, all 80
  src blocks, partition = src % 128). The x16 scale keeps norms in
  e4m3's sweet spot; it is undone exactly via bias*16 + relu + pool/16.
- Device: h = x@W (fp8) on PE, kept in SBUF; out^T[H, dst] accumulates
  DoubleRow fp8 matmuls (2 src blocks per instruction) over 40 block
  pairs, one dst bank at a time, so each bank's post-processing (fused
  relu+bias ACT drain, PE transpose, pooling matmul) overlaps the next
  bank's A-stream. A-stream DMAs round-robin over three DGE queues.
- Each core outputs its partial (pre-softmax) logits [64,2] = FC(its
  pooled partial) + b_fc/8; there is no cross-core communication on
  device. kernel() gathers the 8 partial outputs and unshards (sums +
  log_softmax) on host, like any other sum-sharded output gather.
"""
import numpy as np
import ml_dtypes

N = 10000
E = 640000
F = 128
HD = 128
G = 64
NC = 8
PER = N // NC           # 1250
NPAD = 10240            # 80 blocks of 128 src rows (even, for DoubleRow pairs)
NBLK = NPAD // 128      # 80
BANK = 512
BANKW = [512, 512, 226]
NBANK = 3
ABLK = 16               # src blocks per A-stream DMA
ASCALE = 16.0

BF16 = ml_dtypes.bfloat16
F8 = ml_dtypes.float8_e4m3

def _plan(edge_index, batch_idx):
    src = edge_index[0].astype(np.int64)
    dst = edge_index[1].astype(np.int64)
    loops = np.arange(N, dtype=np.int64)
    src_f = np.concatenate([src, loops])
    dst_f = np.concatenate([dst, loops])

    deg = np.bincount(dst_f, minlength=N).astype(np.float64)
    dinv = 1.0 / np.sqrt(deg)
    wts = dinv[src_f] * dinv[dst_f] * ASCALE

    core_of = dst_f // PER
    A_ship = np.zeros((NC, 128, NBLK * PER), dtype=F8)
    for c in range(NC):
        m = core_of == c
        flat = src_f[m] * PER + (dst_f[m] - c * PER)
        A = np.bincount(flat, weights=wts[m], minlength=NPAD * PER)
        A3 = A.reshape(NBLK, 128, PER).transpose(1, 0, 2)  # [128, NBLK, PER]
        off = 0
        for o0, w in zip((0, 512, 1024), BANKW):
            A_ship[c, :, off:off + NBLK * w] = \
                A3[:, :, o0:o0 + w].reshape(128, NBLK * w).astype(F8)
            off += NBLK * w

    cnt = np.bincount(batch_idx.astype(np.int64), minlength=G).astype(np.float64)
    cnt = np.maximum(cnt, 1.0)
    mp = np.zeros((NC, 1280, G), dtype=np.float64)
    for c in range(NC):
        nodes = np.arange(c * PER, (c + 1) * PER)
        g = batch_idx[nodes].astype(np.int64)
        mp[c, np.arange(PER), g] = 1.0 / (cnt[g] * ASCALE)
    mp = mp.reshape(NC, 10, 128, G)
    mp = np.transpose(mp, (0, 2, 1, 3)).reshape(NC, 128, 10 * G).astype(np.float32)

    return dict(A_ship=A_ship, mpool=mp)


def _build():
    import concourse.bacc as bacc
    import concourse.mybir as mybir
    import concourse.tile as tile

    f32 = mybir.dt.float32
    fp8 = mybir.dt.float8e4
    AF = mybir.ActivationFunctionType
    ALU = mybir.AluOpType
    DR = mybir.MatmulPerfMode.DoubleRow

    nc = bacc.Bacc("TRN2", target_bir_lowering=False, debug=False, num_devices=NC)

    xT = nc.dram_tensor("xT", [F, NPAD], fp8, kind="ExternalInput")
    Wt = nc.dram_tensor("Wt", [F, HD], fp8, kind="ExternalInput")
    Amat = nc.dram_tensor("Amat", [128, NBLK * PER], fp8, kind="ExternalInput")
    bvec = nc.dram_tensor("bvec", [128, 1], f32, kind="ExternalInput")  # 16*b
    Wfc = nc.dram_tensor("Wfc", [HD, 2], f32, kind="ExternalInput")
    bfc = nc.dram_tensor("bfc", [G, 2], f32, kind="ExternalInput")      # b_fc/NC bcast
    idn = nc.dram_tensor("idn", [128, 128], f32, kind="ExternalInput")  # identity
    mpool = nc.dram_tensor("mpool", [128, 10 * G], f32, kind="ExternalInput")
    out = nc.dram_tensor("out", [G, 2], f32, kind="ExternalOutput")

    with tile.TileContext(nc) as tc:
        with tc.tile_pool(name="const", bufs=1) as cp, \
             tc.tile_pool(name="astream", bufs=3) as ap_pool, \
             tc.tile_pool(name="aggp", bufs=1, space="PSUM") as aggp, \
             tc.tile_pool(name="tps", bufs=2, space="PSUM") as tps:

            # ---- constants: W + xT on scalar queue (A-stream owns the
            # sync/gpsimd rings at startup), small consts after xt0 ----
            W_sb = cp.tile([F, HD], fp8)
            nc.scalar.dma_start(W_sb[:], Wt[:])
            XC = 16 * 128           # xT load chunk (16 src blocks)
            xt_tiles = []
            for xc in range(NBLK // 16):
                lo = xc * XC
                hi = min(NPAD, lo + XC)
                xt_t = cp.tile([F, hi - lo], fp8, name=f"xt{xc}")
                nc.scalar.dma_start(xt_t[:], xT[:, lo:hi])
                xt_tiles.append(xt_t)
            bv_sb = cp.tile([128, 1], f32)
            nc.scalar.dma_start(bv_sb[:], bvec[:])
            Wfc_sb = cp.tile([HD, 2], f32)
            nc.scalar.dma_start(Wfc_sb[:], Wfc[:])
            bfc_sb = cp.tile([G, 2], f32)
            nc.scalar.dma_start(bfc_sb[:], bfc[:])
            idn_sb = cp.tile([128, 128], f32)
            nc.scalar.dma_start(idn_sb[:], idn[:])
            mp_sb = cp.tile([128, 10 * G], f32)
            nc.scalar.dma_start(mp_sb[:], mpool[:])

            agg = []
            for bk in range(NBANK):
                agg_t = aggp.tile([128, BANK], f32, tag=f"agg{bk}", name=f"agg{bk}")
                agg.append(agg_t)

            # ---- main: bank 0 interleaves h = x@W group-by-group with its
            # A-tile consumption (PE fills DMA-bound slack with h work);
            # banks 1-2 are pure A streaming ----
            h_sb = cp.tile([128, NBLK * 128], fp8)
            outT_sb = cp.tile([128, 1280], f32)
            hn_sb = cp.tile([128, 1280], f32)
            pp = tps.tile([128, G], f32, tag="pool", name="pp", bufs=1)
            aoff = 0
            # bank 0: scalar ring is busy with W/xT/consts, keep it clear
            dma_plan = {0: [nc.sync, nc.gpsimd],
                        1: [nc.sync, nc.gpsimd, nc.scalar],
                        2: [nc.sync, nc.gpsimd, nc.scalar]}
            dma_i = 0
            NPAIR = NBLK // 2

            def h_group(cb, hps):
                njj = min(4, NBLK - cb)
                hp = hps.tile([128, 512], f32, tag="hps")
                for j in range(njj):
                    ci = cb + j
                    xt_t = xt_tiles[ci // 16]
                    co = (ci % 16) * 128
                    nc.tensor.matmul(
                        hp[:, j * 128:(j + 1) * 128],
                        xt_t[:, co:co + 128],
                        W_sb[:],
                        start=True, stop=True,
                    )
                half = njj * 64
                nc.vector.tensor_copy(
                    h_sb[:, cb * 128:cb * 128 + half], hp[:, :half])
                nc.scalar.copy(
                    h_sb[:, cb * 128 + half:(cb + njj) * 128],
                    hp[:, half:njj * 128])

            with tc.tile_pool(name="hps", bufs=3, space="PSUM") as hps:
                for bk in range(NBANK):
                    w = BANKW[bk]
                    for b0 in range(0, NBLK, ABLK):
                        nb = min(ABLK, NBLK - b0)
                        at = ap_pool.tile([128, ABLK * BANK], fp8, tag="at",
                                          bufs=6)
                        eng = dma_plan[bk][dma_i % len(dma_plan[bk])]
                        dma_i += 1
                        eng.dma_start(
                            at[:, :nb * w],
                            Amat[:, aoff + b0 * w:aoff + (b0 + nb) * w])
                        if bk == 0:
                            # compute h for exactly the blocks this A tile
                            # contracts with, right before consuming it
                            for cb in range(b0, b0 + nb, 4):
                                h_group(cb, hps)
                        for j in range(0, nb, 2):
                            pr = (b0 + j) // 2
                            nc.tensor.matmul(
                                agg[bk][:, :w],
                                h_sb[:, (b0 + j) * 128:(b0 + j + 2) * 128]
                                    .rearrange("p (k m) -> p k m", k=2),
                                at[:, j * w:(j + 2) * w]
                                    .rearrange("p (k n) -> p k n", k=2),
                                start=(pr == 0),
                                stop=(pr == NPAIR - 1),
                                skip_group_check=True,
                                perf_mode=DR,
                            )
                    aoff += NBLK * w
                    # fused bias+relu drain (out^T: bias per-partition, x16)
                    nc.scalar.activation(
                        outT_sb[:, bk * BANK:bk * BANK + w], agg[bk][:, :w],
                        AF.Relu, bias=bv_sb[:, 0:1])
                    if w < BANK and bk * BANK + w < 1280:
                        nc.vector.memset(outT_sb[:, bk * BANK + w:1280], 0.0)
                    t0 = (bk * BANK) // 128
                    t1 = (bk * BANK + w + 127) // 128
                    for t in range(t0, min(t1, 10)):
                        tp = tps.tile([128, 128], f32, tag="tp", bufs=1)
                        nc.tensor.transpose(
                            tp[:], outT_sb[:, t * 128:(t + 1) * 128], idn_sb[:])
                        nc.vector.tensor_copy(
                            hn_sb[:, t * 128:(t + 1) * 128], tp[:])
                        nc.tensor.matmul(
                            pp[:],
                            hn_sb[:, t * 128:(t + 1) * 128],
                            mp_sb[:, t * G:(t + 1) * G],
                            start=(t == 0), stop=(t == 9),
                            skip_group_check=True,
                        )

            pooled_sb = cp.tile([128, G], f32)
            nc.scalar.copy(pooled_sb[:], pp[:])

            # ---- FC partial (+ b_fc/NC), straight to DRAM out ----
            lg = tps.tile([G, 2], f32, tag="pool", name="lg", bufs=1)
            nc.tensor.matmul(lg[:], pooled_sb[:], Wfc_sb[:], start=True, stop=True)
            lpart = cp.tile([G, 2], f32)
            nc.vector.tensor_tensor(lpart[:], lg[:], bfc_sb[:], ALU.add)
            nc.sync.dma_start(out[:], lpart[:])

    nc.compile()
    return nc


def _make_inputs(x, W, b, W_fc, b_fc, p):
    xT = np.zeros((F, NPAD), dtype=F8)
    xT[:, :N] = np.asarray(x, dtype=np.float32).T.astype(F8)
    shared = dict(
        xT=xT,
        Wt=np.asarray(W, dtype=np.float32).astype(F8),
        bvec=(np.asarray(b, dtype=np.float32) * ASCALE).reshape(128, 1).copy(),
        Wfc=np.asarray(W_fc, dtype=np.float32),
        bfc=np.tile(np.asarray(b_fc, dtype=np.float32)[None, :] / NC, (G, 1)),
        idn=np.eye(128, dtype=np.float32),
    )
    in_maps = []
    for c in range(NC):
        m = dict(shared)
        m["Amat"] = p["A_ship"][c]
        m["mpool"] = p["mpool"][c]
        in_maps.append(m)
    return in_maps


def _post(results):
    """Gather/unshard: sum the 8 partial logits, then log_softmax."""
    logits = np.zeros((G, 2), dtype=np.float64)
    for r in results:
        logits += np.asarray(r["out"], dtype=np.float64)
    mx = logits.max(axis=1, keepdims=True)
    e = np.exp(logits - mx)
    ls = logits - mx - np.log(e.sum(axis=1, keepdims=True))
    return ls.astype(np.float32)


def kernel(x, edge_index, batch_idx, W, b, W_fc, b_fc):
    from concourse.bass_utils import run_bass_kernel_spmd

    p = _plan(np.asarray(edge_index), np.asarray(batch_idx))
    nc = _build()
    in_maps = _make_inputs(x, W, b, W_fc, b_fc, p)
    res = run_bass_kernel_spmd(nc, in_maps, core_ids=list(range(NC)))
    return _post(res.results)


# revision 23
# speedup vs baseline: 1.1629x; 1.1392x over previous
"""CrystalGNN (GCNConv + mean-pool + FC + log_softmax) on 8 TRN2 NeuronCores.

Strategy (dst-range partitioned, dense normalized adjacency):
- Core c owns dst nodes [c*1250, (c+1)*1250). The host builds the
  normalized adjacency block A_c[src, dst_local] = 16 * sum over edges
  (incl. self-loops) of dinv[src]*dinv[dst], shipped as fp8-e4m3
  (~12.6MB/core), pre-blocked bank-major (per 512-col PSUM bank, all 80
  src blocks, partition = src % 128). The x16 scale keeps norms in
  e4m3's sweet spot; it is undone exactly via bias*16 + relu + pool/16.
- Device: h = x@W (fp8) on PE, kept in SBUF; out^T[H, dst] accumulates
  DoubleRow fp8 matmuls (2 src blocks per instruction) over 40 block
  pairs, one dst bank at a time, so each bank's post-processing (fused
  relu+bias ACT drain, PE transpose, pooling matmul) overlaps the next
  bank's A-stream. A-stream DMAs round-robin over three DGE queues.
- Each core outputs its partial (pre-softmax) logits [64,2] = FC(its
  pooled partial) + b_fc/8; there is no cross-core communication on
  device. kernel() gathers the 8 partial outputs and unshards (sums +
  log_softmax) on host, like any other sum-sharded output gather.
"""
import numpy as np
import ml_dtypes

N = 10000
E = 640000
F = 128
HD = 128
G = 64
NC = 8
PER = N // NC           # 1250
NPAD = 10240            # 80 blocks of 128 src rows (even, for DoubleRow pairs)
NBLK = NPAD // 128      # 80
BANK = 512
BANKW = [512, 512, 226]
NBANK = 3
ABLK = 16               # src blocks per A-stream DMA
ASCALE = 16.0

BF16 = ml_dtypes.bfloat16
F8 = ml_dtypes.float8_e4m3

def _plan(edge_index, batch_idx):
    src = edge_index[0].astype(np.int64)
    dst = edge_index[1].astype(np.int64)
    loops = np.arange(N, dtype=np.int64)
    src_f = np.concatenate([src, loops])
    dst_f = np.concatenate([dst, loops])

    deg = np.bincount(dst_f, minlength=N).astype(np.float64)
    dinv = 1.0 / np.sqrt(deg)
    wts = dinv[src_f] * dinv[dst_f] * ASCALE

    core_of = dst_f // PER
    A_ship = np.zeros((NC, 128, NBLK * PER), dtype=F8)
    for c in range(NC):
        m = core_of == c
        flat = src_f[m] * PER + (dst_f[m] - c * PER)
        A = np.bincount(flat, weights=wts[m], minlength=NPAD * PER)
        A3 = A.reshape(NBLK, 128, PER).transpose(1, 0, 2)  # [128, NBLK, PER]
        off = 0
        for o0, w in zip((0, 512, 1024), BANKW):
            A_ship[c, :, off:off + NBLK * w] = \
                A3[:, :, o0:o0 + w].reshape(128, NBLK * w).astype(F8)
            off += NBLK * w

    cnt = np.bincount(batch_idx.astype(np.int64), minlength=G).astype(np.float64)
    cnt = np.maximum(cnt, 1.0)
    mp = np.zeros((NC, 1280, G), dtype=np.float64)
    for c in range(NC):
        nodes = np.arange(c * PER, (c + 1) * PER)
        g = batch_idx[nodes].astype(np.int64)
        mp[c, np.arange(PER), g] = 1.0 / (cnt[g] * ASCALE)
    mp = mp.reshape(NC, 10, 128, G)
    mp = np.transpose(mp, (0, 2, 1, 3)).reshape(NC, 128, 10 * G).astype(np.float32)

    return dict(A_ship=A_ship, mpool=mp)


def _build():
    import concourse.bacc as bacc
    import concourse.mybir as mybir
    import concourse.tile as tile

    f32 = mybir.dt.float32
    fp8 = mybir.dt.float8e4
    AF = mybir.ActivationFunctionType
    ALU = mybir.AluOpType
    DR = mybir.MatmulPerfMode.DoubleRow

    nc = bacc.Bacc("TRN2", target_bir_lowering=False, debug=False, num_devices=NC)

    xT = nc.dram_tensor("xT", [F, NPAD], fp8, kind="ExternalInput")
    Wt = nc.dram_tensor("Wt", [F, HD], fp8, kind="ExternalInput")
    Amat = nc.dram_tensor("Amat", [128, NBLK * PER], fp8, kind="ExternalInput")
    bvec = nc.dram_tensor("bvec", [128, 1], f32, kind="ExternalInput")  # 16*b
    Wfc = nc.dram_tensor("Wfc", [HD, 2], f32, kind="ExternalInput")
    bfc = nc.dram_tensor("bfc", [G, 2], f32, kind="ExternalInput")      # b_fc/NC bcast
    idn = nc.dram_tensor("idn", [128, 128], f32, kind="ExternalInput")  # identity
    mpool = nc.dram_tensor("mpool", [128, 10 * G], f32, kind="ExternalInput")
    out = nc.dram_tensor("out", [G, 2], f32, kind="ExternalOutput")

    with tile.TileContext(nc) as tc:
        with tc.tile_pool(name="const", bufs=1) as cp, \
             tc.tile_pool(name="astream", bufs=3) as ap_pool, \
             tc.tile_pool(name="aggp", bufs=1, space="PSUM") as aggp, \
             tc.tile_pool(name="tps", bufs=2, space="PSUM") as tps:

            # ---- constants: W + xT first on sync; small consts on scalar ----
            W_sb = cp.tile([F, HD], fp8)
            nc.sync.dma_start(W_sb[:], Wt[:])
            XC = 16 * 128           # xT load chunk (16 src blocks)
            xt_tiles = []
            for xc in range(NBLK // 16):
                lo = xc * XC
                hi = min(NPAD, lo + XC)
                xt_t = cp.tile([F, hi - lo], fp8, name=f"xt{xc}")
                nc.sync.dma_start(xt_t[:], xT[:, lo:hi])
                xt_tiles.append(xt_t)
            bv_sb = cp.tile([128, 1], f32)
            nc.scalar.dma_start(bv_sb[:], bvec[:])
            Wfc_sb = cp.tile([HD, 2], f32)
            nc.scalar.dma_start(Wfc_sb[:], Wfc[:])
            bfc_sb = cp.tile([G, 2], f32)
            nc.scalar.dma_start(bfc_sb[:], bfc[:])
            idn_sb = cp.tile([128, 128], f32)
            nc.scalar.dma_start(idn_sb[:], idn[:])
            mp_sb = cp.tile([128, 10 * G], f32)
            nc.scalar.dma_start(mp_sb[:], mpool[:])

            agg = []
            for bk in range(NBANK):
                agg_t = aggp.tile([128, BANK], f32, tag=f"agg{bk}", name=f"agg{bk}")
                agg.append(agg_t)

            # ---- main: bank 0 interleaves h = x@W group-by-group with its
            # A-tile consumption (PE fills DMA-bound slack with h work);
            # banks 1-2 are pure A streaming ----
            h_sb = cp.tile([128, NBLK * 128], fp8)
            outT_sb = cp.tile([128, 1280], f32)
            hn_sb = cp.tile([128, 1280], f32)
            pp = tps.tile([128, G], f32, tag="pool", name="pp", bufs=1)
            aoff = 0
            # bank 0: sync ring is busy with W/xT, keep it clear
            dma_plan = {0: [nc.gpsimd, nc.scalar],
                        1: [nc.sync, nc.gpsimd, nc.scalar],
                        2: [nc.sync, nc.gpsimd, nc.scalar]}
            dma_i = 0
            NPAIR = NBLK // 2

            def h_group(cb, hps):
                njj = min(4, NBLK - cb)
                hp = hps.tile([128, 512], f32, tag="hps")
                for j in range(njj):
                    ci = cb + j
                    xt_t = xt_tiles[ci // 16]
                    co = (ci % 16) * 128
                    nc.tensor.matmul(
                        hp[:, j * 128:(j + 1) * 128],
                        xt_t[:, co:co + 128],
                        W_sb[:],
                        start=True, stop=True,
                    )
                half = njj * 64
                nc.vector.tensor_copy(
                    h_sb[:, cb * 128:cb * 128 + half], hp[:, :half])
                nc.scalar.copy(
                    h_sb[:, cb * 128 + half:(cb + njj) * 128],
                    hp[:, half:njj * 128])

            with tc.tile_pool(name="hps", bufs=3, space="PSUM") as hps:
                for bk in range(NBANK):
                    w = BANKW[bk]
                    for b0 in range(0, NBLK, ABLK):
                        nb = min(ABLK, NBLK - b0)
                        at = ap_pool.tile([128, ABLK * BANK], fp8, tag="at",
                                          bufs=6)
                        eng = dma_plan[bk][dma_i % len(dma_plan[bk])]
                        dma_i += 1
                        eng.dma_start(
                            at[:, :nb * w],
                            Amat[:, aoff + b0 * w:aoff + (b0 + nb) * w])
                        if bk == 0:
                            # compute h for exactly the blocks this A tile
                            # contracts with, right before consuming it
                            for cb in range(b0, b0 + nb, 4):
                                h_group(cb, hps)
                        for j in range(0, nb, 2):
                            pr = (b0 + j) // 2
                            nc.tensor.matmul(
                                agg[bk][:, :w],
                                h_sb[:, (b0 + j) * 128:(b0 + j + 2) * 128]
                                    .rearrange("p (k m) -> p k m", k=2),
                                at[:, j * w:(j + 2) * w]
                                    .rearrange("p (k n) -> p k n", k=2),
                                start=(pr == 0),
                                stop=(pr == NPAIR - 1),
                                skip_group_check=True,
                                perf_mode=DR,
                            )
                    aoff += NBLK * w
                    # fused bias+relu drain (out^T: bias per-partition, x16)
                    nc.scalar.activation(
                        outT_sb[:, bk * BANK:bk * BANK + w], agg[bk][:, :w],
                        AF.Relu, bias=bv_sb[:, 0:1])
                    if w < BANK and bk * BANK + w < 1280:
                        nc.vector.memset(outT_sb[:, bk * BANK + w:1280], 0.0)
                    t0 = (bk * BANK) // 128
                    t1 = (bk * BANK + w + 127) // 128
                    for t in range(t0, min(t1, 10)):
                        tp = tps.tile([128, 128], f32, tag="tp", bufs=1)
                        nc.tensor.transpose(
                            tp[:], outT_sb[:, t * 128:(t + 1) * 128], idn_sb[:])
                        nc.vector.tensor_copy(
                            hn_sb[:, t * 128:(t + 1) * 128], tp[:])
                        nc.tensor.matmul(
                            pp[:],
                            hn_sb[:, t * 128:(t + 1) * 128],
                            mp_sb[:, t * G:(t + 1) * G],
                            start=(t == 0), stop=(t == 9),
                            skip_group_check=True,
                        )

            pooled_sb = cp.tile([128, G], f32)
            nc.scalar.copy(pooled_sb[:], pp[:])

            # ---- FC partial (+ b_fc/NC), straight to DRAM out ----
            lg = tps.tile([G, 2], f32, tag="pool", name="lg", bufs=1)
            nc.tensor.matmul(lg[:], pooled_sb[:], Wfc_sb[:], start=True, stop=True)
            lpart = cp.tile([G, 2], f32)
            nc.vector.tensor_tensor(lpart[:], lg[:], bfc_sb[:], ALU.add)
            nc.sync.dma_start(out[:], lpart[:])

    nc.compile()
    return nc


def _make_inputs(x, W, b, W_fc, b_fc, p):
    xT = np.zeros((F, NPAD), dtype=F8)
    xT[:, :N] = np.asarray(x, dtype=np.float32).T.astype(F8)
    shared = dict(
        xT=xT,
        Wt=np.asarray(W, dtype=np.float32).astype(F8),
        bvec=(np.asarray(b, dtype=np.float32) * ASCALE).reshape(128, 1).copy(),
        Wfc=np.asarray(W_fc, dtype=np.float32),
        bfc=np.tile(np.asarray(b_fc, dtype=np.float32)[None, :] / NC, (G, 1)),
        idn=np.eye(128, dtype=np.float32),
    )
    in_maps = []
    for c in range(NC):
        m = dict(shared)
        m["Amat"] = p["A_ship"][c]
        m["mpool"] = p["mpool"][c]
        in_maps.append(m)
    return in_maps


def _post(results):
    """Gather/unshard: sum the 8 partial logits, then log_softmax."""
    logits = np.zeros((G, 2), dtype=np.float64)
    for r in results:
        logits += np.asarray(r["out"], dtype=np.float64)
    mx = logits.max(axis=1, keepdims=True)
    e = np.exp(logits - mx)
    ls = logits - mx - np.log(e.sum(axis=1, keepdims=True))
    return ls.astype(np.float32)


def kernel(x, edge_index, batch_idx, W, b, W_fc, b_fc):
    from concourse.bass_utils import run_bass_kernel_spmd

    p = _plan(np.asarray(edge_index), np.asarray(batch_idx))
    nc = _build()
    in_maps = _make_inputs(x, W, b, W_fc, b_fc, p)
    res = run_bass_kernel_spmd(nc, in_maps, core_ids=list(range(NC)))
    return _post(res.results)
